# revision 15
# baseline (speedup 1.0000x reference)
"""DeepSet encoder (phi MLP -> sum/max pool -> rho MLP) as a Trainium2 Bass kernel.

Sharding: data-parallel over the batch dim. 64 samples -> 8 cores x 8 samples.
Weights are replicated on every core; no cross-core communication.

On-chip layout is feature-major ("transposed"): activations live as
[feature_partition, set_free] tiles so that
  - matmul contraction (over features) is on the partition dim,
  - the bias is a per-partition scalar (free on ScalarE's activation op),
  - sum/max pooling over the set dim is a free-axis reduction
    (sum comes for free via activation's accum_out).
The host pre-transposes x to [B, D_IN, N] and casts inputs to bf16.

Self-contained: only relies on the system-installed concourse/bass stack.
"""

import sys

import numpy as np

for _p in ("/opt/trn_rl_repo",):
    if _p not in sys.path:
        sys.path.insert(0, _p)

import ml_dtypes  # noqa: E402

import concourse.bass as bass  # noqa: E402,F401
import concourse.mybir as mybir  # noqa: E402
import concourse.tile as tile  # noqa: E402
from concourse import bacc  # noqa: E402
from concourse.bass_utils import run_bass_kernel_spmd  # noqa: E402

# 16-bit compute dtype: fp16 runs the PE at the same 1 cycle/row as bf16 but
# carries 10 mantissa bits instead of 8. All intermediates here are O(100) max,
# far inside fp16 range, so fp16 is a free 4x accuracy win over bf16.
BF16 = mybir.dt.float16
FP32 = mybir.dt.float32
NP_BF16 = np.float16

B, N, D_IN, D_H = 64, 512, 512, 1024
N_CORES = 8
BL = B // N_CORES  # samples per core
P = 128
K1 = D_IN // P  # phi1 contraction tiles (4)
K2 = D_H // P  # phi2/rho2 contraction tiles & D_H output tiles (8)
KR1 = 2 * D_H // P  # rho1 contraction tiles (16)

RELU = mybir.ActivationFunctionType.Relu
AX_X = mybir.AxisListType.X
OP_MAX = mybir.AluOpType.max


def build_program() -> bacc.Bacc:
    nc = bacc.Bacc("TRN2", target_bir_lowering=False, debug=False, num_devices=N_CORES)

    # all staged host-side into the exact SBUF tile layouts so every DMA is
    # contiguous per partition (large descriptor runs):
    #   xt[b, p, k, n] = x[b, n, k*128+p];  w*[p, ko, h] = W[ko*128+p, h]
    xt_d = nc.dram_tensor("xt", [BL, P, K1, N], BF16, kind="ExternalInput").ap()
    w1_d = nc.dram_tensor("w1", [P, K1, D_H], BF16, kind="ExternalInput").ap()
    w2_d = nc.dram_tensor("w2", [P, K2, D_H], BF16, kind="ExternalInput").ap()
    wr1_d = nc.dram_tensor("wr1", [P, KR1, D_H], BF16, kind="ExternalInput").ap()
    wr2_d = nc.dram_tensor("wr2", [P, K2, D_H], BF16, kind="ExternalInput").ap()
    # biases staged on host as [P, n_tiles]: b_sb[p, m] = b[m*128 + p]
    b1_d = nc.dram_tensor("b1", [P, K2], FP32, kind="ExternalInput").ap()
    b2_d = nc.dram_tensor("b2", [P, K2], FP32, kind="ExternalInput").ap()
    br1_d = nc.dram_tensor("br1", [P, K2], FP32, kind="ExternalInput").ap()
    br2_d = nc.dram_tensor("br2", [P, K2], FP32, kind="ExternalInput").ap()
    # out[p, m, s] = r2[m*128 + p, s]  (feature-major, host transposes back)
    out_d = nc.dram_tensor("out", [P, K2, BL], FP32, kind="ExternalOutput").ap()

    with tile.TileContext(nc) as tc:
        with (
            tc.tile_pool(name="const", bufs=1) as cpool,
            tc.tile_pool(name="xt", bufs=3) as xtpool,
            tc.tile_pool(name="h1", bufs=2) as h1pool,
            tc.tile_pool(name="h2", bufs=4) as h2pool,
            tc.tile_pool(name="ps", bufs=4, space="PSUM") as pspool,
            tc.tile_pool(name="ps2", bufs=4, space="PSUM") as ps2pool,
        ):
            # --- PE warm-up ---
            # The PE clock sits at 1.2GHz (HAM-throttled) until ~3.4us of
            # sustained activity. Burn that window on dummy matmuls over a
            # zeroed scratch tile while the startup DMAs are in flight, so
            # the real matmuls run at 2.4GHz from the first one.
            warm_sb = cpool.tile([P, N], BF16)
            nc.gpsimd.memset(warm_sb[:], 0.0)
            for i in range(8):
                wps = pspool.tile([P, N], FP32, tag="ps", name=f"warm{i}")
                nc.tensor.matmul(wps[:], warm_sb[:, 0:P], warm_sb[:], start=True, stop=True)

            # --- persistent SBUF state ---
            # startup-critical DMAs first: the sync sequencer issues one
            # DIRECT2D per ~0.6us, so issue order = time order. Interleave
            # per-k parts of xt[0] and w1 so the first matmuls can begin
            # after ~400KB instead of ~4MB; everything else queues behind.
            w1_sb = cpool.tile([P, K1, D_H], BF16)
            xt0_sb = xtpool.tile([P, K1, N], BF16, tag="xt", name="xt0")
            xt1_sb = xtpool.tile([P, K1, N], BF16, tag="xt", name="xt1")
            for k in range(K1):
                nc.sync.dma_start(xt0_sb[:, k, :], xt_d[0, :, k, :])
                nc.sync.dma_start(w1_sb[:, k, :], w1_d[:, k, :])
                if k == 2:
                    # hoisted so sample 1's input is resident before phi1(1);
                    # costs one issue slot (~0.6us) on the last two k parts.
                    nc.sync.dma_start(xt1_sb[:], xt_d[1])
            b1_sb = cpool.tile([P, K2], FP32)
            nc.sync.dma_start(b1_sb[:], b1_d)
            w2_sb = cpool.tile([P, K2, D_H], BF16)
            nc.sync.dma_start(w2_sb[:], w2_d)
            b2_sb = cpool.tile([P, K2], FP32)
            nc.sync.dma_start(b2_sb[:], b2_d)

            pooled = cpool.tile([P, KR1, BL], FP32)  # [0:K2]=sum, [K2:]=max
            pooled_bf = cpool.tile([P, KR1, BL], BF16)
            r1_sb = cpool.tile([P, K2, BL], BF16)
            out_sb = cpool.tile([P, K2, BL], FP32)

            def phi1(b):
                if b == 0:
                    xt_sb = xt0_sb
                elif b == 1:
                    xt_sb = xt1_sb
                else:
                    xt_sb = xtpool.tile([P, K1, N], BF16, tag="xt", name=f"xt{b}")
                    nc.sync.dma_start(xt_sb[:], xt_d[b])
                h1_sb = h1pool.tile([P, K2, N], BF16, tag="h1", name=f"h1_{b}")
                for m in range(K2):
                    ps = pspool.tile([P, N], FP32, tag="ps", name=f"ps1_{b}_{m}")
                    for k in range(K1):
                        nc.tensor.matmul(
                            ps[:],
                            w1_sb[:, k, m * P : (m + 1) * P],
                            xt_sb[:, k, :],
                            start=(k == 0),
                            stop=(k == K1 - 1),
                        )
                    nc.scalar.activation(
                        h1_sb[:, m, :], ps[:], RELU, bias=b1_sb[:, m : m + 1], scale=1.0
                    )
                return h1_sb

            def phi2(b, h1_sb):
                for m in range(K2):
                    ps = pspool.tile([P, N], FP32, tag="ps", name=f"ps2_{b}_{m}")
                    for k in range(K2):
                        nc.tensor.matmul(
                            ps[:],
                            w2_sb[:, k, m * P : (m + 1) * P],
                            h1_sb[:, k, :],
                            start=(k == 0),
                            stop=(k == K2 - 1),
                        )
                    h2_sb = h2pool.tile([P, N], BF16, tag="h2", name=f"h2_{b}_{m}")
                    # relu(psum + bias) -> h2 tile; sum over set dim lands in
                    # pooled[:, m, b] via the activation accumulator.
                    nc.scalar.activation(
                        h2_sb[:],
                        ps[:],
                        RELU,
                        bias=b2_sb[:, m : m + 1],
                        scale=1.0,
                        accum_out=pooled[:, m, b : b + 1],
                    )
                    nc.vector.tensor_reduce(
                        pooled[:, K2 + m, b : b + 1], h2_sb[:], axis=AX_X, op=OP_MAX
                    )
                    if b == BL - 1:
                        # last sample: this feature tile is complete -> cast it
                        # now so rho1's matmuls can chase the phi2 epilogue.
                        nc.vector.tensor_copy(pooled_bf[:, m, :], pooled[:, m, :])
                        nc.vector.tensor_copy(
                            pooled_bf[:, K2 + m, :], pooled[:, K2 + m, :]
                        )

            # software pipeline: phi1(b+1) is emitted before phi2(b) so the PE
            # never waits on the phi1->phi2 evacuation inside one sample.
            prev_h1 = None
            for b in range(BL):
                h1_sb = phi1(b)
                if prev_h1 is not None:
                    phi2(b - 1, prev_h1)
                prev_h1 = h1_sb
            phi2(BL - 1, prev_h1)

            # --- rho MLP over the 8 pooled vectors (feature-major, N=8) ---
            wr1_sb = cpool.tile([P, KR1, D_H], BF16)
            nc.sync.dma_start(wr1_sb[:], wr1_d)
            wr2_sb = cpool.tile([P, K2, D_H], BF16)
            nc.sync.dma_start(wr2_sb[:], wr2_d)
            br1_sb = cpool.tile([P, K2], FP32)
            nc.sync.dma_start(br1_sb[:], br1_d)
            br2_sb = cpool.tile([P, K2], FP32)
            nc.sync.dma_start(br2_sb[:], br2_d)

            for m in range(K2):
                ps = ps2pool.tile([P, BL], FP32, tag="ps2", name=f"psr1_{m}")
                for k in range(KR1):
                    nc.tensor.matmul(
                        ps[:],
                        wr1_sb[:, k, m * P : (m + 1) * P],
                        pooled_bf[:, k, :],
                        start=(k == 0),
                        stop=(k == KR1 - 1),
                    )
                nc.scalar.activation(
                    r1_sb[:, m, :], ps[:], RELU, bias=br1_sb[:, m : m + 1], scale=1.0
                )
            for m in range(K2):
                ps = ps2pool.tile([P, BL], FP32, tag="ps2", name=f"psr2_{m}")
                for k in range(K2):
                    nc.tensor.matmul(
                        ps[:],
                        wr2_sb[:, k, m * P : (m + 1) * P],
                        r1_sb[:, k, :],
                        start=(k == 0),
                        stop=(k == K2 - 1),
                    )
                nc.scalar.activation(
                    out_sb[:, m, :], ps[:], RELU, bias=br2_sb[:, m : m + 1], scale=1.0
                )
                if m == K2 // 2 - 1:
                    # first half of the output leaves while rho2 finishes
                    nc.sync.dma_start(out_d[:, : K2 // 2], out_sb[:, : K2 // 2])
            nc.sync.dma_start(out_d[:, K2 // 2 :], out_sb[:, K2 // 2 :])

    return nc


_CACHE: dict = {}


def get_compiled() -> bacc.Bacc:
    if "nc" not in _CACHE:
        nc = build_program()
        nc.compile()
        _CACHE["nc"] = nc
    return _CACHE["nc"]


def stage_inputs(x, W_phi1, b_phi1, W_phi2, b_phi2, W_rho1, b_rho1, W_rho2, b_rho2):
    """Host-side staging: transpose x, cast to bf16, reshape biases."""

    def wtile(a):
        # [KO*P, H] -> [P, KO, H] with w[p, ko, h] = W[ko*P + p, h]
        a = np.asarray(a, np.float32).astype(NP_BF16)
        ko = a.shape[0] // P
        return np.ascontiguousarray(a.reshape(ko, P, -1).transpose(1, 0, 2))

    def bias(a):
        # [n_tiles*P] -> [P, n_tiles] with b_sb[p, m] = b[m*P + p]
        return np.ascontiguousarray(np.asarray(a, np.float32).reshape(-1, P).T)

    # x[b, n, d] -> xt[b, p, k, n] = x[b, n, k*P+p]
    xt = np.asarray(x, np.float32).astype(NP_BF16)
    xt = np.ascontiguousarray(xt.reshape(B, N, K1, P).transpose(0, 3, 2, 1))
    shared = {
        "w1": wtile(W_phi1),
        "w2": wtile(W_phi2),
        "wr1": wtile(W_rho1),
        "wr2": wtile(W_rho2),
        "b1": bias(b_phi1),
        "b2": bias(b_phi2),
        "br1": bias(b_rho1),
        "br2": bias(b_rho2),
    }
    in_maps = []
    for c in range(N_CORES):
        m = dict(shared)
        m["xt"] = np.ascontiguousarray(xt[c * BL : (c + 1) * BL])
        in_maps.append(m)
    return in_maps


def gather_output(results) -> np.ndarray:
    # per-core out: [P, K2, BL] with out[p, m, s] = r2[m*128+p, s]
    parts = []
    for c in range(N_CORES):
        o = np.asarray(results[c]["out"], np.float32)  # [P, K2, BL]
        parts.append(o.transpose(2, 1, 0).reshape(BL, D_H))  # [BL, D_H]
    return np.concatenate(parts, axis=0)


def run(trace: bool = False, **inputs):
    nc = get_compiled()
    in_maps = stage_inputs(**inputs)
    res = run_bass_kernel_spmd(nc, in_maps, core_ids=list(range(N_CORES)), trace=trace)
    return gather_output(res.results), res


def kernel(**inputs) -> np.ndarray:
    out, _ = run(trace=False, **inputs)
    return out


# revision 16
# speedup vs baseline: 1.0095x; 1.0095x over previous
"""DeepSet encoder (phi MLP -> sum/max pool -> rho MLP) as a Trainium2 Bass kernel.

Sharding: data-parallel over the batch dim. 64 samples -> 8 cores x 8 samples.
Weights are replicated on every core; no cross-core communication.

On-chip layout is feature-major ("transposed"): activations live as
[feature_partition, set_free] tiles so that
  - matmul contraction (over features) is on the partition dim,
  - the bias is a per-partition scalar (free on ScalarE's activation op),
  - sum/max pooling over the set dim is a free-axis reduction
    (sum comes for free via activation's accum_out).
The host pre-transposes x to [B, D_IN, N] and casts inputs to bf16.

Self-contained: only relies on the system-installed concourse/bass stack.
"""

import sys

import numpy as np

for _p in ("/opt/trn_rl_repo",):
    if _p not in sys.path:
        sys.path.insert(0, _p)

import ml_dtypes  # noqa: E402

import concourse.bass as bass  # noqa: E402,F401
import concourse.mybir as mybir  # noqa: E402
import concourse.tile as tile  # noqa: E402
from concourse import bacc  # noqa: E402
from concourse.bass_utils import run_bass_kernel_spmd  # noqa: E402

# 16-bit compute dtype: fp16 runs the PE at the same 1 cycle/row as bf16 but
# carries 10 mantissa bits instead of 8. All intermediates here are O(100) max,
# far inside fp16 range, so fp16 is a free 4x accuracy win over bf16.
BF16 = mybir.dt.float16
FP32 = mybir.dt.float32
NP_BF16 = np.float16

B, N, D_IN, D_H = 64, 512, 512, 1024
N_CORES = 8
BL = B // N_CORES  # samples per core
P = 128
K1 = D_IN // P  # phi1 contraction tiles (4)
K2 = D_H // P  # phi2/rho2 contraction tiles & D_H output tiles (8)
KR1 = 2 * D_H // P  # rho1 contraction tiles (16)

RELU = mybir.ActivationFunctionType.Relu
AX_X = mybir.AxisListType.X
OP_MAX = mybir.AluOpType.max


def build_program() -> bacc.Bacc:
    nc = bacc.Bacc("TRN2", target_bir_lowering=False, debug=False, num_devices=N_CORES)

    # all staged host-side into the exact SBUF tile layouts so every DMA is
    # contiguous per partition (large descriptor runs):
    #   xt[b, p, k, n] = x[b, n, k*128+p];  w*[p, ko, h] = W[ko*128+p, h]
    xt_d = nc.dram_tensor("xt", [BL, P, K1, N], BF16, kind="ExternalInput").ap()
    w1_d = nc.dram_tensor("w1", [P, K1, D_H], BF16, kind="ExternalInput").ap()
    w2_d = nc.dram_tensor("w2", [P, K2, D_H], BF16, kind="ExternalInput").ap()
    wr1_d = nc.dram_tensor("wr1", [P, KR1, D_H], BF16, kind="ExternalInput").ap()
    wr2_d = nc.dram_tensor("wr2", [P, K2, D_H], BF16, kind="ExternalInput").ap()
    # biases staged on host as [P, n_tiles]: b_sb[p, m] = b[m*128 + p]
    b1_d = nc.dram_tensor("b1", [P, K2], FP32, kind="ExternalInput").ap()
    b2_d = nc.dram_tensor("b2", [P, K2], FP32, kind="ExternalInput").ap()
    br1_d = nc.dram_tensor("br1", [P, K2], FP32, kind="ExternalInput").ap()
    br2_d = nc.dram_tensor("br2", [P, K2], FP32, kind="ExternalInput").ap()
    # out[p, m, s] = r2[m*128 + p, s]  (feature-major, host transposes back)
    out_d = nc.dram_tensor("out", [P, K2, BL], FP32, kind="ExternalOutput").ap()

    with tile.TileContext(nc) as tc:
        with (
            tc.tile_pool(name="const", bufs=1) as cpool,
            tc.tile_pool(name="xt", bufs=3) as xtpool,
            tc.tile_pool(name="h1", bufs=2) as h1pool,
            tc.tile_pool(name="h2", bufs=4) as h2pool,
            tc.tile_pool(name="ps", bufs=4, space="PSUM") as pspool,
            tc.tile_pool(name="ps2", bufs=4, space="PSUM") as ps2pool,
        ):
            # --- PE warm-up ---
            # The PE clock sits at 1.2GHz (HAM-throttled) until ~3.4us of
            # sustained activity. Burn that window on dummy matmuls over a
            # zeroed scratch tile while the startup DMAs are in flight, so
            # the real matmuls run at 2.4GHz from the first one.
            warm_sb = cpool.tile([P, N], BF16)
            nc.gpsimd.memset(warm_sb[:], 0.0)
            for i in range(8):
                wps = pspool.tile([P, N], FP32, tag="ps", name=f"warm{i}")
                nc.tensor.matmul(wps[:], warm_sb[:, 0:P], warm_sb[:], start=True, stop=True)

            # --- persistent SBUF state ---
            # startup-critical DMAs first: the sync sequencer issues one
            # DIRECT2D per ~0.6us, so issue order = time order. Interleave
            # per-k parts of xt[0] and w1 so the first matmuls can begin
            # after ~400KB instead of ~4MB; everything else queues behind.
            w1_sb = cpool.tile([P, K1, D_H], BF16)
            xt0_sb = xtpool.tile([P, K1, N], BF16, tag="xt", name="xt0")
            xt1_sb = xtpool.tile([P, K1, N], BF16, tag="xt", name="xt1")
            for h in range(2):
                ks = slice(h * K1 // 2, (h + 1) * K1 // 2)
                nc.sync.dma_start(xt0_sb[:, ks, :], xt_d[0, :, ks, :])
                nc.sync.dma_start(w1_sb[:, ks, :], w1_d[:, ks, :])
            b1_sb = cpool.tile([P, K2], FP32)
            nc.sync.dma_start(b1_sb[:], b1_d)
            nc.sync.dma_start(xt1_sb[:], xt_d[1])
            w2_sb = cpool.tile([P, K2, D_H], BF16)
            nc.sync.dma_start(w2_sb[:], w2_d)
            b2_sb = cpool.tile([P, K2], FP32)
            nc.sync.dma_start(b2_sb[:], b2_d)

            pooled = cpool.tile([P, KR1, BL], FP32)  # [0:K2]=sum, [K2:]=max
            pooled_bf = cpool.tile([P, KR1, BL], BF16)
            r1_sb = cpool.tile([P, K2, BL], BF16)
            out_sb = cpool.tile([P, K2, BL], FP32)

            def phi1(b):
                if b == 0:
                    xt_sb = xt0_sb
                elif b == 1:
                    xt_sb = xt1_sb
                else:
                    xt_sb = xtpool.tile([P, K1, N], BF16, tag="xt", name=f"xt{b}")
                    nc.sync.dma_start(xt_sb[:], xt_d[b])
                h1_sb = h1pool.tile([P, K2, N], BF16, tag="h1", name=f"h1_{b}")
                for m in range(K2):
                    ps = pspool.tile([P, N], FP32, tag="ps", name=f"ps1_{b}_{m}")
                    for k in range(K1):
                        nc.tensor.matmul(
                            ps[:],
                            w1_sb[:, k, m * P : (m + 1) * P],
                            xt_sb[:, k, :],
                            start=(k == 0),
                            stop=(k == K1 - 1),
                        )
                    nc.scalar.activation(
                        h1_sb[:, m, :], ps[:], RELU, bias=b1_sb[:, m : m + 1], scale=1.0
                    )
                return h1_sb

            def phi2(b, h1_sb):
                for m in range(K2):
                    ps = pspool.tile([P, N], FP32, tag="ps", name=f"ps2_{b}_{m}")
                    for k in range(K2):
                        nc.tensor.matmul(
                            ps[:],
                            w2_sb[:, k, m * P : (m + 1) * P],
                            h1_sb[:, k, :],
                            start=(k == 0),
                            stop=(k == K2 - 1),
                        )
                    h2_sb = h2pool.tile([P, N], BF16, tag="h2", name=f"h2_{b}_{m}")
                    # relu(psum + bias) -> h2 tile; sum over set dim lands in
                    # pooled[:, m, b] via the activation accumulator.
                    nc.scalar.activation(
                        h2_sb[:],
                        ps[:],
                        RELU,
                        bias=b2_sb[:, m : m + 1],
                        scale=1.0,
                        accum_out=pooled[:, m, b : b + 1],
                    )
                    nc.vector.tensor_reduce(
                        pooled[:, K2 + m, b : b + 1], h2_sb[:], axis=AX_X, op=OP_MAX
                    )
                    if b == BL - 1:
                        # last sample: this feature tile is complete -> cast it
                        # now so rho1's matmuls can chase the phi2 epilogue.
                        nc.vector.tensor_copy(pooled_bf[:, m, :], pooled[:, m, :])
                        nc.vector.tensor_copy(
                            pooled_bf[:, K2 + m, :], pooled[:, K2 + m, :]
                        )

            # software pipeline: phi1(b+1) is emitted before phi2(b) so the PE
            # never waits on the phi1->phi2 evacuation inside one sample.
            prev_h1 = None
            for b in range(BL):
                h1_sb = phi1(b)
                if prev_h1 is not None:
                    phi2(b - 1, prev_h1)
                prev_h1 = h1_sb
            phi2(BL - 1, prev_h1)

            # --- rho MLP over the 8 pooled vectors (feature-major, N=8) ---
            wr1_sb = cpool.tile([P, KR1, D_H], BF16)
            nc.sync.dma_start(wr1_sb[:], wr1_d)
            wr2_sb = cpool.tile([P, K2, D_H], BF16)
            nc.sync.dma_start(wr2_sb[:], wr2_d)
            br1_sb = cpool.tile([P, K2], FP32)
            nc.sync.dma_start(br1_sb[:], br1_d)
            br2_sb = cpool.tile([P, K2], FP32)
            nc.sync.dma_start(br2_sb[:], br2_d)

            for m in range(K2):
                ps = ps2pool.tile([P, BL], FP32, tag="ps2", name=f"psr1_{m}")
                for k in range(KR1):
                    nc.tensor.matmul(
                        ps[:],
                        wr1_sb[:, k, m * P : (m + 1) * P],
                        pooled_bf[:, k, :],
                        start=(k == 0),
                        stop=(k == KR1 - 1),
                    )
                nc.scalar.activation(
                    r1_sb[:, m, :], ps[:], RELU, bias=br1_sb[:, m : m + 1], scale=1.0
                )
            for m in range(K2):
                ps = ps2pool.tile([P, BL], FP32, tag="ps2", name=f"psr2_{m}")
                for k in range(K2):
                    nc.tensor.matmul(
                        ps[:],
                        wr2_sb[:, k, m * P : (m + 1) * P],
                        r1_sb[:, k, :],
                        start=(k == 0),
                        stop=(k == K2 - 1),
                    )
                nc.scalar.activation(
                    out_sb[:, m, :], ps[:], RELU, bias=br2_sb[:, m : m + 1], scale=1.0
                )
                if m == K2 // 2 - 1:
                    # first half of the output leaves while rho2 finishes
                    nc.sync.dma_start(out_d[:, : K2 // 2], out_sb[:, : K2 // 2])
            nc.sync.dma_start(out_d[:, K2 // 2 :], out_sb[:, K2 // 2 :])

    return nc


_CACHE: dict = {}


def get_compiled() -> bacc.Bacc:
    if "nc" not in _CACHE:
        nc = build_program()
        nc.compile()
        _CACHE["nc"] = nc
    return _CACHE["nc"]


def stage_inputs(x, W_phi1, b_phi1, W_phi2, b_phi2, W_rho1, b_rho1, W_rho2, b_rho2):
    """Host-side staging: transpose x, cast to bf16, reshape biases."""

    def wtile(a):
        # [KO*P, H] -> [P, KO, H] with w[p, ko, h] = W[ko*P + p, h]
        a = np.asarray(a, np.float32).astype(NP_BF16)
        ko = a.shape[0] // P
        return np.ascontiguousarray(a.reshape(ko, P, -1).transpose(1, 0, 2))

    def bias(a):
        # [n_tiles*P] -> [P, n_tiles] with b_sb[p, m] = b[m*P + p]
        return np.ascontiguousarray(np.asarray(a, np.float32).reshape(-1, P).T)

    # x[b, n, d] -> xt[b, p, k, n] = x[b, n, k*P+p]
    xt = np.asarray(x, np.float32).astype(NP_BF16)
    xt = np.ascontiguousarray(xt.reshape(B, N, K1, P).transpose(0, 3, 2, 1))
    shared = {
        "w1": wtile(W_phi1),
        "w2": wtile(W_phi2),
        "wr1": wtile(W_rho1),
        "wr2": wtile(W_rho2),
        "b1": bias(b_phi1),
        "b2": bias(b_phi2),
        "br1": bias(b_rho1),
        "br2": bias(b_rho2),
    }
    in_maps = []
    for c in range(N_CORES):
        m = dict(shared)
        m["xt"] = np.ascontiguousarray(xt[c * BL : (c + 1) * BL])
        in_maps.append(m)
    return in_maps


def gather_output(results) -> np.ndarray:
    # per-core out: [P, K2, BL] with out[p, m, s] = r2[m*128+p, s]
    parts = []
    for c in range(N_CORES):
        o = np.asarray(results[c]["out"], np.float32)  # [P, K2, BL]
        parts.append(o.transpose(2, 1, 0).reshape(BL, D_H))  # [BL, D_H]
    return np.concatenate(parts, axis=0)


def run(trace: bool = False, **inputs):
    nc = get_compiled()
    in_maps = stage_inputs(**inputs)
    res = run_bass_kernel_spmd(nc, in_maps, core_ids=list(range(N_CORES)), trace=trace)
    return gather_output(res.results), res


def kernel(**inputs) -> np.ndarray:
    out, _ = run(trace=False, **inputs)
    return out


# revision 20
# speedup vs baseline: 1.0155x; 1.0060x over previous
"""DeepSet encoder (phi MLP -> sum/max pool -> rho MLP) as a Trainium2 Bass kernel.

Sharding: data-parallel over the batch dim. 64 samples -> 8 cores x 8 samples.
Weights are replicated on every core; no cross-core communication.

On-chip layout is feature-major ("transposed"): activations live as
[feature_partition, set_free] tiles so that
  - matmul contraction (over features) is on the partition dim,
  - the bias is a per-partition scalar (free on ScalarE's activation op),
  - sum/max pooling over the set dim is a free-axis reduction
    (sum comes for free via activation's accum_out).
The host pre-transposes x to [B, D_IN, N] and casts inputs to bf16.

Self-contained: only relies on the system-installed concourse/bass stack.
"""

import sys

import numpy as np

for _p in ("/opt/trn_rl_repo",):
    if _p not in sys.path:
        sys.path.insert(0, _p)

import ml_dtypes  # noqa: E402

import concourse.bass as bass  # noqa: E402,F401
import concourse.mybir as mybir  # noqa: E402
import concourse.tile as tile  # noqa: E402
from concourse import bacc  # noqa: E402
from concourse.bass_utils import run_bass_kernel_spmd  # noqa: E402

# 16-bit compute dtype: fp16 runs the PE at the same 1 cycle/row as bf16 but
# carries 10 mantissa bits instead of 8. All intermediates here are O(100) max,
# far inside fp16 range, so fp16 is a free 4x accuracy win over bf16.
BF16 = mybir.dt.float16
FP32 = mybir.dt.float32
NP_BF16 = np.float16

B, N, D_IN, D_H = 64, 512, 512, 1024
N_CORES = 8
BL = B // N_CORES  # samples per core
P = 128
K1 = D_IN // P  # phi1 contraction tiles (4)
K2 = D_H // P  # phi2/rho2 contraction tiles & D_H output tiles (8)
KR1 = 2 * D_H // P  # rho1 contraction tiles (16)

RELU = mybir.ActivationFunctionType.Relu
AX_X = mybir.AxisListType.X
OP_MAX = mybir.AluOpType.max


def build_program() -> bacc.Bacc:
    nc = bacc.Bacc("TRN2", target_bir_lowering=False, debug=False, num_devices=N_CORES)

    # all staged host-side into the exact SBUF tile layouts so every DMA is
    # contiguous per partition (large descriptor runs):
    #   xt[b, p, k, n] = x[b, n, k*128+p];  w*[p, ko, h] = W[ko*128+p, h]
    xt_d = nc.dram_tensor("xt", [BL, P, K1, N], BF16, kind="ExternalInput").ap()
    w1_d = nc.dram_tensor("w1", [P, K1, D_H], BF16, kind="ExternalInput").ap()
    w2_d = nc.dram_tensor("w2", [P, K2, D_H], BF16, kind="ExternalInput").ap()
    wr1_d = nc.dram_tensor("wr1", [P, KR1, D_H], BF16, kind="ExternalInput").ap()
    wr2_d = nc.dram_tensor("wr2", [P, K2, D_H], BF16, kind="ExternalInput").ap()
    # biases staged on host as [P, n_tiles]: b_sb[p, m] = b[m*128 + p]
    b1_d = nc.dram_tensor("b1", [P, K2], FP32, kind="ExternalInput").ap()
    b2_d = nc.dram_tensor("b2", [P, K2], FP32, kind="ExternalInput").ap()
    br1_d = nc.dram_tensor("br1", [P, K2], FP32, kind="ExternalInput").ap()
    br2_d = nc.dram_tensor("br2", [P, K2], FP32, kind="ExternalInput").ap()
    # out[p, m, s] = r2[m*128 + p, s]  (feature-major, host transposes back)
    out_d = nc.dram_tensor("out", [P, K2, BL], FP32, kind="ExternalOutput").ap()

    with tile.TileContext(nc) as tc:
        with (
            tc.tile_pool(name="const", bufs=1) as cpool,
            tc.tile_pool(name="xt", bufs=3) as xtpool,
            tc.tile_pool(name="h1", bufs=2) as h1pool,
            tc.tile_pool(name="h2", bufs=4) as h2pool,
            tc.tile_pool(name="ps", bufs=8, space="PSUM") as pspool,
        ):
            # --- PE warm-up ---
            # The PE clock sits at 1.2GHz (HAM-throttled) until ~3.4us of
            # sustained activity. Burn that window on dummy matmuls over a
            # zeroed scratch tile while the startup DMAs are in flight, so
            # the real matmuls run at 2.4GHz from the first one.
            warm_sb = cpool.tile([P, N], BF16)
            nc.gpsimd.memset(warm_sb[:], 0.0)
            for i in range(8):
                wps = pspool.tile([P, N], FP32, tag="ps", name=f"warm{i}")
                nc.tensor.matmul(wps[:], warm_sb[:, 0:P], warm_sb[:], start=True, stop=True)

            # --- persistent SBUF state ---
            # startup-critical DMAs first: the sync sequencer issues one
            # DIRECT2D per ~0.6us, so issue order = time order. Interleave
            # per-k parts of xt[0] and w1 so the first matmuls can begin
            # after ~400KB instead of ~4MB; everything else queues behind.
            w1_sb = cpool.tile([P, K1, D_H], BF16)
            xt0_sb = xtpool.tile([P, K1, N], BF16, tag="xt", name="xt0")
            xt1_sb = xtpool.tile([P, K1, N], BF16, tag="xt", name="xt1")
            for h in range(2):
                ks = slice(h * K1 // 2, (h + 1) * K1 // 2)
                nc.sync.dma_start(xt0_sb[:, ks, :], xt_d[0, :, ks, :])
                nc.sync.dma_start(w1_sb[:, ks, :], w1_d[:, ks, :])
            b1_sb = cpool.tile([P, K2], FP32)
            nc.sync.dma_start(b1_sb[:], b1_d)
            nc.sync.dma_start(xt1_sb[:], xt_d[1])
            w2_sb = cpool.tile([P, K2, D_H], BF16)
            nc.sync.dma_start(w2_sb[:], w2_d)
            b2_sb = cpool.tile([P, K2], FP32)
            nc.sync.dma_start(b2_sb[:], b2_d)

            pooled = cpool.tile([P, KR1, BL], FP32)  # [0:K2]=sum, [K2:]=max
            pooled_bf = cpool.tile([P, KR1, BL], BF16)
            r1_sb = cpool.tile([P, K2, BL], BF16)
            out_sb = cpool.tile([P, K2, BL], FP32)

            def phi1(b):
                if b == 0:
                    xt_sb = xt0_sb
                elif b == 1:
                    xt_sb = xt1_sb
                else:
                    xt_sb = xtpool.tile([P, K1, N], BF16, tag="xt", name=f"xt{b}")
                    nc.sync.dma_start(xt_sb[:], xt_d[b])
                h1_sb = h1pool.tile([P, K2, N], BF16, tag="h1", name=f"h1_{b}")
                for m in range(K2):
                    ps = pspool.tile([P, N], FP32, tag="ps", name=f"ps1_{b}_{m}")
                    for k in range(K1):
                        nc.tensor.matmul(
                            ps[:],
                            w1_sb[:, k, m * P : (m + 1) * P],
                            xt_sb[:, k, :],
                            start=(k == 0),
                            stop=(k == K1 - 1),
                        )
                    nc.scalar.activation(
                        h1_sb[:, m, :], ps[:], RELU, bias=b1_sb[:, m : m + 1], scale=1.0
                    )
                return h1_sb

            def phi2(b, h1_sb):
                for m in range(K2):
                    ps = pspool.tile([P, N], FP32, tag="ps", name=f"ps2_{b}_{m}")
                    for k in range(K2):
                        nc.tensor.matmul(
                            ps[:],
                            w2_sb[:, k, m * P : (m + 1) * P],
                            h1_sb[:, k, :],
                            start=(k == 0),
                            stop=(k == K2 - 1),
                        )
                    h2_sb = h2pool.tile([P, N], BF16, tag="h2", name=f"h2_{b}_{m}")
                    # relu(psum + bias) -> h2 tile; sum over set dim lands in
                    # pooled[:, m, b] via the activation accumulator.
                    nc.scalar.activation(
                        h2_sb[:],
                        ps[:],
                        RELU,
                        bias=b2_sb[:, m : m + 1],
                        scale=1.0,
                        accum_out=pooled[:, m, b : b + 1],
                    )
                    if b == BL - 1:
                        # last sample: the sum feature tile is complete as soon
                        # as the ACT accumulator lands -> cast it before the
                        # max reduce so rho1's sum-half matmuls can start.
                        nc.vector.tensor_copy(pooled_bf[:, m, :], pooled[:, m, :])
                    nc.vector.tensor_reduce(
                        pooled[:, K2 + m, b : b + 1], h2_sb[:], axis=AX_X, op=OP_MAX
                    )
                    if b == BL - 1:
                        nc.vector.tensor_copy(
                            pooled_bf[:, K2 + m, :], pooled[:, K2 + m, :]
                        )

            # software pipeline: phi1(b+1) is emitted before phi2(b) so the PE
            # never waits on the phi1->phi2 evacuation inside one sample.
            prev_h1 = None
            for b in range(BL):
                h1_sb = phi1(b)
                if prev_h1 is not None:
                    phi2(b - 1, prev_h1)
                prev_h1 = h1_sb
            phi2(BL - 1, prev_h1)

            # --- rho MLP over the 8 pooled vectors (feature-major, N=8) ---
            wr1_sb = cpool.tile([P, KR1, D_H], BF16)
            nc.sync.dma_start(wr1_sb[:], wr1_d)
            wr2_sb = cpool.tile([P, K2, D_H], BF16)
            nc.sync.dma_start(wr2_sb[:], wr2_d)
            br1_sb = cpool.tile([P, K2], FP32)
            nc.sync.dma_start(br1_sb[:], br1_d)
            br2_sb = cpool.tile([P, K2], FP32)
            nc.sync.dma_start(br2_sb[:], br2_d)

            # rho1 in two half-accumulations over all 8 m-tiles: the sum-half
            # (k=0..7) only needs the ACT accumulators, so its matmuls chase
            # the phi2 epilogue while the max reduces are still draining.
            ps_r1 = []
            for m in range(K2):
                ps = pspool.tile([P, BL], FP32, tag="ps", name=f"psr1_{m}")
                ps_r1.append(ps)
                for k in range(K2):
                    nc.tensor.matmul(
                        ps[:],
                        wr1_sb[:, k, m * P : (m + 1) * P],
                        pooled_bf[:, k, :],
                        start=(k == 0),
                        stop=False,
                    )
            for m in range(K2):
                ps = ps_r1[m]
                for k in range(K2, KR1):
                    nc.tensor.matmul(
                        ps[:],
                        wr1_sb[:, k, m * P : (m + 1) * P],
                        pooled_bf[:, k, :],
                        start=False,
                        stop=(k == KR1 - 1),
                    )
                nc.scalar.activation(
                    r1_sb[:, m, :], ps[:], RELU, bias=br1_sb[:, m : m + 1], scale=1.0
                )
            for m in range(K2):
                ps = pspool.tile([P, BL], FP32, tag="ps", name=f"psr2_{m}")
                for k in range(K2):
                    nc.tensor.matmul(
                        ps[:],
                        wr2_sb[:, k, m * P : (m + 1) * P],
                        r1_sb[:, k, :],
                        start=(k == 0),
                        stop=(k == K2 - 1),
                    )
                nc.scalar.activation(
                    out_sb[:, m, :], ps[:], RELU, bias=br2_sb[:, m : m + 1], scale=1.0
                )
                if m == K2 // 2 - 1:
                    # first half of the output leaves while rho2 finishes
                    nc.sync.dma_start(out_d[:, : K2 // 2], out_sb[:, : K2 // 2])
            nc.sync.dma_start(out_d[:, K2 // 2 :], out_sb[:, K2 // 2 :])

    return nc


_CACHE: dict = {}


def get_compiled() -> bacc.Bacc:
    if "nc" not in _CACHE:
        nc = build_program()
        nc.compile()
        _CACHE["nc"] = nc
    return _CACHE["nc"]


def stage_inputs(x, W_phi1, b_phi1, W_phi2, b_phi2, W_rho1, b_rho1, W_rho2, b_rho2):
    """Host-side staging: transpose x, cast to bf16, reshape biases."""

    def wtile(a):
        # [KO*P, H] -> [P, KO, H] with w[p, ko, h] = W[ko*P + p, h]
        a = np.asarray(a, np.float32).astype(NP_BF16)
        ko = a.shape[0] // P
        return np.ascontiguousarray(a.reshape(ko, P, -1).transpose(1, 0, 2))

    def bias(a):
        # [n_tiles*P] -> [P, n_tiles] with b_sb[p, m] = b[m*P + p]
        return np.ascontiguousarray(np.asarray(a, np.float32).reshape(-1, P).T)

    # x[b, n, d] -> xt[b, p, k, n] = x[b, n, k*P+p]
    xt = np.asarray(x, np.float32).astype(NP_BF16)
    xt = np.ascontiguousarray(xt.reshape(B, N, K1, P).transpose(0, 3, 2, 1))
    shared = {
        "w1": wtile(W_phi1),
        "w2": wtile(W_phi2),
        "wr1": wtile(W_rho1),
        "wr2": wtile(W_rho2),
        "b1": bias(b_phi1),
        "b2": bias(b_phi2),
        "br1": bias(b_rho1),
        "br2": bias(b_rho2),
    }
    in_maps = []
    for c in range(N_CORES):
        m = dict(shared)
        m["xt"] = np.ascontiguousarray(xt[c * BL : (c + 1) * BL])
        in_maps.append(m)
    return in_maps


def gather_output(results) -> np.ndarray:
    # per-core out: [P, K2, BL] with out[p, m, s] = r2[m*128+p, s]
    parts = []
    for c in range(N_CORES):
        o = np.asarray(results[c]["out"], np.float32)  # [P, K2, BL]
        parts.append(o.transpose(2, 1, 0).reshape(BL, D_H))  # [BL, D_H]
    return np.concatenate(parts, axis=0)


def run(trace: bool = False, **inputs):
    nc = get_compiled()
    in_maps = stage_inputs(**inputs)
    res = run_bass_kernel_spmd(nc, in_maps, core_ids=list(range(N_CORES)), trace=trace)
    return gather_output(res.results), res


def kernel(**inputs) -> np.ndarray:
    out, _ = run(trace=False, **inputs)
    return out
